# revision 1
# baseline (speedup 1.0000x reference)
"""FFT conv block (rfft2 -> per-channel complex multiply -> irfft2) on 8 trn2
cores — v3: radix-2 W-axis transforms + bf16 middle stages.

Math per (b,c) image [256, 256] (sharded over channels, 8 ch/core):
  T1: rfft over H via matmul, split into even/odd w columns:
      Y1e[we, kh], Y1o[wo, kh]   (kh 0..128, f32r)
  T2: radix-2 fft over W: A = DFT128(even w), TB = twiddled DFT128(odd w)
      (twiddles folded into the constant matrices); butterfly on DVE/Pool:
      Y2lo = A + TB, Y2hi = A - TB  -> bf16
  wm: Yw = Y2 * Weff (bf16, DVE 2x, weights broadcast over images)
  S/D: S = Yw_lo + Yw_hi, D = Yw_lo - Yw_hi (radix-2 DIF for the inverse)
  T3: Z_even = S @ E3e, Z_odd = D @ E3o (bf16 matmuls, blocked e|o layout)
  T4: y = A4^T Zr + B4^T Zi + a4n (x) znyq, reading Z through an interleaving
      AP so y comes out w-contiguous (f32r matmuls)
Weff remaps the reference's [kh_full, kw_half] weights onto the
[kh_half, kw_full] quarter-plane.  Verified vs reference: ~3.6e-3 rel rms
(bf16-dominated), tolerance 2e-2.
"""
import sys
sys.path.insert(0, "/opt/trn_rl_repo")
import numpy as np
import ml_dtypes

B, C, H, W = 16, 64, 256, 256
KHF = H // 2 + 1          # 129
KP = 130                  # kh padded (even, fp32r ISA rule)
N_CORES = 8
NC_LOC = C // N_CORES     # 8 channels per core
NB = B                    # 16 batch images per channel
G = 8                     # supergroup size (images of same channel)


def _consts_v3(g=G):
    f32 = np.float32
    h = np.arange(H)[:, None]
    kh = np.arange(KHF)[None, :]
    th = 2 * np.pi * h * kh / H                      # [H, KHF]
    z = np.zeros((H, 1))
    M1 = np.concatenate([np.cos(th), z, -np.sin(th), z], axis=1)  # [256, 260]

    kw = np.arange(128)[None, :]
    we = np.arange(128)[:, None]
    cosE = np.cos(2 * np.pi * (2 * we) * kw / W)     # [we, kw]
    sinE = np.sin(2 * np.pi * (2 * we) * kw / W)
    cosO = np.cos(2 * np.pi * (2 * we + 1) * kw / W)
    sinO = np.sin(2 * np.pi * (2 * we + 1) * kw / W)

    kw2 = np.arange(128)[:, None]
    m = np.arange(128)[None, :]
    Ce = np.cos(2 * np.pi * kw2 * (2 * m) / W) / W
    Se = np.sin(2 * np.pi * kw2 * (2 * m) / W) / W
    Co = np.cos(2 * np.pi * kw2 * (2 * m + 1) / W) / W
    So = np.sin(2 * np.pi * kw2 * (2 * m + 1) / W) / W

    kh2 = np.arange(KHF)[:, None]
    hp = np.arange(H)[None, :]
    t4 = 2 * np.pi * kh2 * hp / H
    alpha = np.where((kh2 == 0) | (kh2 == H // 2), 1.0, 2.0)
    A4 = alpha * np.cos(t4) / H                      # [129, 256]
    B4 = -alpha * np.sin(t4) / H
    B4[0, :] = 0.0
    a4n = A4[128, :]                                 # [256]
    # masked nyquist stationary: a4ng[c, gg*256 + m] = (gg==c) * a4n[m]
    a4ng = np.zeros((g, g * 256))
    for gg in range(g):
        a4ng[gg, gg * 256:(gg + 1) * 256] = a4n
    bf = ml_dtypes.bfloat16
    return dict(
        M1=M1.astype(f32),
        cosE=cosE.astype(f32), sinE=sinE.astype(f32), nsinE=(-sinE).astype(f32),
        cosO=cosO.astype(f32), sinO=sinO.astype(f32), nsinO=(-sinO).astype(f32),
        Ce=Ce.astype(f32).astype(bf), Se=Se.astype(f32).astype(bf),
        nSe=(-Se).astype(f32).astype(bf), Co=Co.astype(f32).astype(bf),
        So=So.astype(f32).astype(bf), nSo=(-So).astype(f32).astype(bf),
        A4m=A4[0:128].astype(f32), B4m=B4[0:128].astype(f32),
        a4ng=a4ng.astype(f32),
    )


def _w_eff(wr, wi):
    """wr, wi: [256(kh), 129(kw)] reference layout -> W_eff [129(kh), 256(kw)]."""
    w = wr.astype(np.float64) + 1j * wi.astype(np.float64)
    kh = np.arange(KHF)
    khc = (H - kh) % H
    eff = np.empty((KHF, W), dtype=np.complex128)
    eff[:, 0:W // 2 + 1] = w[0:KHF, :]
    for kwv in range(W // 2 + 1, W):
        eff[:, kwv] = np.conj(w[khc, W - kwv])
    for col in (0, W // 2):
        eff[:, col] = 0.5 * (w[kh, col] + np.conj(w[khc, col]))
    return eff                                        # [129(kh), 256(kw)] complex


def build_nc(nc_loc=NC_LOC, nb=NB, g=G, repeat=1):
    import concourse.mybir as mybir
    import concourse.tile as tile
    from concourse import bacc
    from contextlib import ExitStack

    f32, f32r = mybir.dt.float32, mybir.dt.float32r
    bf16 = mybir.dt.bfloat16
    n_img = nb * nc_loc
    npairs = g // 2
    CN = _consts_v3(g)

    nc = bacc.Bacc("TRN2", target_bir_lowering=False)
    x_d = nc.dram_tensor("x", [n_img, H, W], f32r, kind="ExternalInput")
    w_d = nc.dram_tensor("w", [nc_loc, 3, 2, 128, KP], bf16, kind="ExternalInput")
    y_d = nc.dram_tensor("y", [n_img, H, W], f32, kind="ExternalOutput")

    dconst = {k: nc.inline_tensor(v, f"c_{k}") for k, v in CN.items()}

    with tile.TileContext(nc) as tc, ExitStack() as es:
        cpool = es.enter_context(tc.tile_pool(name="const", bufs=1))
        stage = es.enter_context(tc.tile_pool(name="stage", bufs=2))
        wpool = es.enter_context(tc.tile_pool(name="wpool", bufs=1))
        xrp = es.enter_context(tc.tile_pool(name="xr", bufs=2))
        y1p = es.enter_context(tc.tile_pool(name="y1", bufs=3))
        y2p = es.enter_context(tc.tile_pool(name="y2", bufs=2))
        ywp = es.enter_context(tc.tile_pool(name="yw", bufs=2))
        sdp = es.enter_context(tc.tile_pool(name="sd", bufs=2))
        zsb = es.enter_context(tc.tile_pool(name="zsb", bufs=3))
        znp = es.enter_context(tc.tile_pool(name="znp", bufs=2))
        ysb = es.enter_context(tc.tile_pool(name="ysb", bufs=3))
        ps1 = es.enter_context(tc.tile_pool(name="ps1", bufs=2, space="PSUM"))
        ps2 = es.enter_context(tc.tile_pool(name="ps2", bufs=1, space="PSUM"))
        ps3 = es.enter_context(tc.tile_pool(name="ps3", bufs=2, space="PSUM"))
        ps4 = es.enter_context(tc.tile_pool(name="ps4", bufs=2, space="PSUM"))

        def load_const(name, dt_):
            """Direct DMA from the inline const into SBUF. f32r tiles are
            bit-identical to f32 so the DMA goes through a bitcast view."""
            src = dconst[name]
            rows, cols = CN[name].shape
            src_is_bf = CN[name].dtype == ml_dtypes.bfloat16
            tiles = []
            nch = (rows + 127) // 128
            for ch in range(nch):
                r0, r1 = ch * 128, min((ch + 1) * 128, rows)
                t = cpool.tile([r1 - r0, cols], dt_, name=f"c_{name}{ch}")
                out_ap = t[:, :] if src_is_bf else t[:, :].bitcast(f32)
                nc.sync.dma_start(out=out_ap, in_=src[r0:r1, :])
                tiles.append(t)
            return tiles if nch > 1 else tiles[0]

        m1t = load_const("M1", f32r)               # 2 x [128, 260]
        cosEt = load_const("cosE", f32r)           # [128, 128] each
        sinEt = load_const("sinE", f32r)
        nsinEt = load_const("nsinE", f32r)
        cosOt = load_const("cosO", f32r)
        sinOt = load_const("sinO", f32r)
        nsinOt = load_const("nsinO", f32r)
        Cet = load_const("Ce", bf16)
        Set = load_const("Se", bf16)
        nSet = load_const("nSe", bf16)
        Cot = load_const("Co", bf16)
        Sot = load_const("So", bf16)
        nSot = load_const("nSo", bf16)
        a4mt = load_const("A4m", f32r)             # [128, 256]
        b4mt = load_const("B4m", f32r)
        a4ngt_flat = load_const("a4ng", f32r)      # [G, G*256]

        # weights: per channel [128(kw), 3(kind), 2(kwc), KP] bf16.
        # Issued on the ACT queue so the first x loads (SP) aren't delayed.
        wt = []
        for cl in range(nc_loc):
            t = wpool.tile([128, 3, 2, KP], bf16, name=f"w{cl}")
            nc.scalar.dma_start(
                out=t, in_=w_d[cl].rearrange("k c p f -> p k c f"))
            wt.append(t)

        r_, i_ = slice(0, KP), slice(KP, 2 * KP)

        def emit_AB(cl, sg0):
            """Phase A (per pair T1+T2+butterfly) + B (wmul, S/D, nyquist)."""
            y2r = y2p.tile([128, 2, g, KP], bf16, name="y2r")
            y2i = y2p.tile([128, 2, g, KP], bf16, name="y2i")
            for pr in range(npairs):
                img0 = cl * nb + sg0 + 2 * pr
                xr = xrp.tile([128, 2, 2, W], f32r, name="xr")
                nc.sync.dma_start(
                    out=xr,
                    in_=x_d[img0:img0 + 2].rearrange(
                        "i (c p) w -> p i c w", c=2))
                # T1: even/odd w columns; Y1 = [128(w/2), 2j, 260]
                y1e = y1p.tile([128, 2, 2 * KP], f32r, name="y1e")
                y1o = y1p.tile([128, 2, 2 * KP], f32r, name="y1o")
                for j in range(2):
                    for par, y1t in ((0, y1e), (1, y1o)):
                        t1 = ps1.tile([128, 2 * KP], f32, name="t1ps")
                        nc.tensor.matmul(
                            t1, xr[:, j, 0, par::2], m1t[0],
                            start=True, stop=False)
                        nc.tensor.matmul(
                            t1, xr[:, j, 1, par::2], m1t[1],
                            start=False, stop=True)
                        nc.scalar.copy(out=y1t[:, j, :], in_=t1)
                # T2 radix-2 over w.  PE accumulates Y2lo = A + TB fully in
                # PSUM (twiddles folded into the odd-column consts); ACT
                # stages lo -> bf16 SBUF; DVE computes hi = lo - 2*TB via
                # scalar_tensor_tensor (only TB read from PSUM).  r and i
                # phases share two PSUM slots (tags) to stay in 8 banks.
                from concourse import mybir as _mb
                sl = slice(2 * pr, 2 * pr + 2)
                for half, y2t in ((0, y2r), (1, y2i)):
                    p_, q_ = (r_, i_) if half == 0 else (i_, r_)
                    sE2 = sinEt if half == 0 else nsinEt
                    sO2 = sinOt if half == 0 else nsinOt
                    lo = ps2.tile([128, 2, KP], f32, name="lops")
                    tb = ps2.tile([128, 2, KP], f32, name="tbps")
                    nc.tensor.matmul(tb, cosOt, y1o[:, :, p_], start=True, stop=False)
                    nc.tensor.matmul(tb, sO2, y1o[:, :, q_], start=False, stop=True)
                    nc.tensor.matmul(lo, cosEt, y1e[:, :, p_], start=True, stop=False)
                    nc.tensor.matmul(lo, sE2, y1e[:, :, q_], start=False, stop=False)
                    nc.tensor.matmul(lo, cosOt, y1o[:, :, p_], start=False, stop=False)
                    nc.tensor.matmul(lo, sO2, y1o[:, :, q_], start=False, stop=True)
                    nc.scalar.copy(out=y2t[:, 0, sl, :], in_=lo)
                    nc.vector.scalar_tensor_tensor(
                        out=y2t[:, 1, sl, :], in0=tb, scalar=-2.0,
                        in1=y2t[:, 0, sl, :],
                        op0=_mb.AluOpType.mult, op1=_mb.AluOpType.add)
            # ---- phase B: wmul (bf16, DVE 2x) + S/D at sg level ----
            wr_b = wt[cl][:, 0, :, :].unsqueeze(2).broadcast_to([128, 2, g, KP])
            wi_b = wt[cl][:, 1, :, :].unsqueeze(2).broadcast_to([128, 2, g, KP])
            nwi_b = wt[cl][:, 2, :, :].unsqueeze(2).broadcast_to([128, 2, g, KP])
            ta = ywp.tile([128, 2, g, KP], bf16, name="ta")
            tb = ywp.tile([128, 2, g, KP], bf16, name="tb")
            tb2 = ywp.tile([128, 2, g, KP], bf16, name="tb2")
            ywr = ywp.tile([128, 2, g, KP], bf16, name="ywr")
            ywi = ywp.tile([128, 2, g, KP], bf16, name="ywi")
            nc.vector.tensor_mul(ta, y2r, wr_b)
            nc.gpsimd.tensor_mul(tb, y2i, nwi_b)
            nc.vector.tensor_add(ywr, ta, tb)
            nc.vector.tensor_mul(ta, y2r, wi_b)
            nc.gpsimd.tensor_mul(tb2, y2i, wr_b)
            nc.vector.tensor_add(ywi, ta, tb2)
            sr = sdp.tile([128, g, KP], bf16, name="sr")
            si = sdp.tile([128, g, KP], bf16, name="si")
            dr = sdp.tile([128, g, KP], bf16, name="dr")
            di = sdp.tile([128, g, KP], bf16, name="di")
            nc.vector.tensor_add(sr, ywr[:, 0], ywr[:, 1])
            nc.vector.tensor_sub(dr, ywr[:, 0], ywr[:, 1])
            nc.gpsimd.tensor_add(si, ywi[:, 0], ywi[:, 1])
            nc.vector.tensor_sub(di, ywi[:, 0], ywi[:, 1])
            return dict(sr=sr, si=si, dr=dr, di=di, cl=cl, sg0=sg0)

        def emit_C(st):
            """Phase C: nyquist + per image T3; per pair T4 + store."""
            sr, si, dr, di = st["sr"], st["si"], st["dr"], st["di"]
            cl, sg0 = st["cl"], st["sg0"]
            # nyquist row kh=128: zn = [g, 2(e|o), 128]
            zn = ps1.tile([g, 2, 128], f32, name="znps", tag="t1ps")
            nc.tensor.matmul(zn[:, 0], sr[:, :, 128], Cet, start=True, stop=False)
            nc.tensor.matmul(zn[:, 0], si[:, :, 128], nSet, start=False, stop=True)
            nc.tensor.matmul(zn[:, 1], dr[:, :, 128], Cot, start=True, stop=False)
            nc.tensor.matmul(zn[:, 1], di[:, :, 128], nSot, start=False, stop=True)
            znr = znp.tile([g, 2, 128], f32r, name="znr")
            nc.scalar.copy(out=znr, in_=zn)
            for pr in range(npairs):
                zzp = zsb.tile([128, 2, 2, 2, 128], f32r, name="zzp")
                for j in range(2):
                    gg = 2 * pr + j
                    kmain = slice(0, 128)
                    z2 = ps3.tile([128, 2, 2, 128], f32, name="z2ps")
                    nc.tensor.matmul(z2[:, 0, 0], sr[:, gg, kmain], Cet, start=True, stop=False)
                    nc.tensor.matmul(z2[:, 0, 0], si[:, gg, kmain], nSet, start=False, stop=True)
                    nc.tensor.matmul(z2[:, 0, 1], dr[:, gg, kmain], Cot, start=True, stop=False)
                    nc.tensor.matmul(z2[:, 0, 1], di[:, gg, kmain], nSot, start=False, stop=True)
                    nc.tensor.matmul(z2[:, 1, 0], si[:, gg, kmain], Cet, start=True, stop=False)
                    nc.tensor.matmul(z2[:, 1, 0], sr[:, gg, kmain], Set, start=False, stop=True)
                    nc.tensor.matmul(z2[:, 1, 1], di[:, gg, kmain], Cot, start=True, stop=False)
                    nc.tensor.matmul(z2[:, 1, 1], dr[:, gg, kmain], Sot, start=False, stop=True)
                    if j == 0:
                        nc.scalar.copy(out=zzp[:, j], in_=z2)
                    else:
                        nc.vector.tensor_copy(out=zzp[:, j], in_=z2)
                # T4: pair-batched; moving reads Z interleaved (m,blk swap)
                yt = ysb.tile([128, 2, 2, W], f32, name="yt")
                znv = znr[:, :, :].transpose([0, 2, 1])  # [g, 128, 2]
                zr_v = zzp[:, :, 0, :, :].transpose([0, 1, 3, 2])
                zi_v = zzp[:, :, 1, :, :].transpose([0, 1, 3, 2])
                for hc in range(2):
                    cols = slice(hc * 128, (hc + 1) * 128)
                    yp = ps4.tile([128, 2, W], f32, name="yps")
                    nc.tensor.matmul(yp, a4mt[:, cols], zr_v, start=True, stop=False)
                    nc.tensor.matmul(yp, b4mt[:, cols], zi_v, start=False, stop=False)
                    for j in range(2):
                        gg = 2 * pr + j
                        a4sel = a4ngt_flat.rearrange(
                            "p (i f) -> p i f", i=g)[:, gg, cols]
                        nc.tensor.matmul(yp[:, j], a4sel, znv,
                                         start=False, stop=(j == 1))
                    if hc == 0:
                        nc.scalar.copy(out=yt[:, :, hc, :], in_=yp)
                    else:
                        nc.vector.tensor_copy(out=yt[:, :, hc, :], in_=yp)
                img0 = cl * nb + sg0 + 2 * pr
                nc.scalar.dma_start(
                    out=y_d[img0:img0 + 2].rearrange(
                        "i (c p) w -> p i c w", c=2), in_=yt)

        # software pipeline: emit C of sg N after A+B of sg N+1 so the
        # in-order PE queue always has independent work ahead of the
        # S/D-dependent T3 matmuls.
        sgs = [(cl, sg0)
               for _rep in range(repeat)
               for cl in range(nc_loc)
               for sg0 in range(0, nb, g)]
        pend = None
        for cl, sg0 in sgs:
            st = emit_AB(cl, sg0)
            if pend is not None:
                emit_C(pend)
            pend = st
        emit_C(pend)
    nc.compile()
    return nc


def _prep_weights(w_real, w_imag, core, nc_loc=NC_LOC):
    """-> [nc_loc, 3, 2, 128, KP] bf16 (kind: Wr, Wi, -Wi; kwc chunks)."""
    warr = np.zeros((nc_loc, 3, 2, 128, KP), ml_dtypes.bfloat16)
    for cl in range(nc_loc):
        eff = _w_eff(w_real[0, core * nc_loc + cl], w_imag[0, core * nc_loc + cl])
        effT = eff.T                        # [256(kw), 129(kh)]
        for k, arr in enumerate([effT.real, effT.imag, -effT.imag]):
            a = np.zeros((2, 128, KP), np.float32)
            a[:, :, 0:KHF] = arr.reshape(2, 128, KHF).astype(np.float32)
            warr[cl, k] = a.astype(ml_dtypes.bfloat16)
    return warr


def _prep_core_inputs(x, w_real, w_imag, core):
    cs = slice(core * NC_LOC, (core + 1) * NC_LOC)
    xc = np.ascontiguousarray(x[:, cs].transpose(1, 0, 2, 3)).reshape(
        B * NC_LOC, H, W).astype(np.float32)
    warr = _prep_weights(w_real, w_imag, core)
    return {"x": xc, "w": warr}


_NC_CACHE = {}


def kernel(x, w_real, w_imag):
    from concourse.bass_utils import run_bass_kernel_spmd
    x = np.asarray(x); w_real = np.asarray(w_real); w_imag = np.asarray(w_imag)
    key = "full"
    if key not in _NC_CACHE:
        _NC_CACHE[key] = build_nc()
    nc = _NC_CACHE[key]
    in_maps = [_prep_core_inputs(x, w_real, w_imag, i) for i in range(N_CORES)]
    res = run_bass_kernel_spmd(nc, in_maps, core_ids=list(range(N_CORES)))
    outs = []
    for i in range(N_CORES):
        yc = res.results[i]["y"].reshape(NC_LOC, B, H, W).transpose(1, 0, 2, 3)
        outs.append(yc)
    return np.concatenate(outs, axis=1)

